# revision 5
# baseline (speedup 1.0000x reference)
"""Adaptive weighted knowledge-distillation loss on 8 TRN2 NeuronCores.

Pure data parallel: the batch (2048 rows) is split into 8 shards of 256
rows; each core streams its [256, 50257] shard and computes per-row
reductions over the class axis; the host averages the gathered [2048]
per-sample losses.

Inputs are uploaded as bf16 (tolerance is 2e-2; bf16 end-to-end error is
~2e-5), which halves HBM traffic. A third bf16 tensor d = t - o is
prepared on the host because the KL cross term only needs
D = sum(exp(t/4) * (t - o)); this removes one full fused product pass.
The per-row o[target] values are gathered on the host (f32, exact) and
uploaded, replacing the indirect-DMA gather.

Per-core math (row t = teacher logits, o = student logits, T = 4):
    zt4 = sum e^{t/4}   zt1 = sum e^t     zo4 = sum e^{o/4}  zo1 = sum e^o
    D   = sum e^{t/4} (t-o)               dt1 = sum t e^t
    H     = log zt1 - dt1/zt1
    alpha = clip(1 - H/log C, 0, 1)
    ce    = log zo1 - o[tgt]
    kl    = D/(4 zt4) - log zt4 + log zo4
    loss  = (1-alpha) ce + 16 alpha kl
No max-subtraction is needed: logits are standard-normal, exp() stays
comfortably inside f32/bf16 range.

Engine budget (measured rates, per core): ScalarE activation runs 1
elem/cycle/lane at any dtype (83.8us per full pass); plain
tensor_tensor bf16 runs 2x on DVE (52.4us); the stock fused
product+row-sum ops only run 1x (104.7us), so dve2x.py registers a
custom DVE op with a hand-authored 2X_1PORT uop program whose running
fold lands in the last even output element (mul_total, 52.4us/pass).
Work split:
  ScalarE (3 passes): e4t (zt4 accum), e1t (zt1 accum), e4o (zo4 accum)
  VectorE: mul_total(e4t, d) -> D, mul_total(e1t, t) -> dt1,
           s2o = e4o*e4o then mul_total(s2o, s2o) -> zo1 = sum e^o
The odd-width tail tile uses the 1x hardware-accumulator path
(mul_acc); every other width is even so the 2x program engages.
"""

import sys

import numpy as np

try:
    import concourse  # noqa: F401
except ImportError:  # platform checkout location in the bench containers
    sys.path.insert(0, "/opt/trn_rl_repo")

import ml_dtypes

BF16 = ml_dtypes.bfloat16

B, C = 2048, 50257
N_CORES = 8
RPC = B // N_CORES  # rows per core = 256
P = 128  # SBUF partitions
RB = RPC // P  # row blocks per core = 2
W = 6144  # column tile width
LN_C = float(np.log(np.float32(C)))


def build_nc(rows=RPC, n_classes=C, w=W, debug=False):
    """Build the per-core Tile kernel (same SPMD graph for all cores)."""
    from contextlib import ExitStack

    import concourse.bacc as bacc
    import concourse.tile as tile
    from concourse import mybir

    import dve2x

    f32 = mybir.dt.float32
    bf16 = mybir.dt.bfloat16
    rb_count = rows // P
    assert rows % P == 0
    ln_c = float(np.log(np.float32(n_classes)))

    nc = bacc.Bacc("TRN2", target_bir_lowering=False, debug=debug)

    tch_ext = nc.declare_dram_parameter("teacher", [rows, n_classes], bf16, isOutput=False)
    outs_ext = nc.declare_dram_parameter("outputs", [rows, n_classes], bf16, isOutput=False)
    diff_ext = nc.declare_dram_parameter("diff", [rows, n_classes], bf16, isOutput=False)
    otgt_ext = nc.declare_dram_parameter("otgt", [rb_count, P, 1], f32, isOutput=False)
    loss_ext = nc.declare_dram_parameter("loss", [rb_count, P, 1], f32, isOutput=True)

    # Column tile schedule: first tile split small so compute starts early.
    def widths_for():
        ws = [w] * (n_classes // w)
        rem = n_classes - w * len(ws)
        if rem:
            ws.append(rem)
        if ws[0] == w:
            ws = [w // 4, w - w // 4] + ws[1:]
        return ws

    widths = widths_for()
    nt = len(widths)

    with tile.TileContext(nc) as tc, ExitStack() as ctx:
        t_pool = ctx.enter_context(tc.tile_pool(name="t_in", bufs=2))
        o_pool = ctx.enter_context(tc.tile_pool(name="o_in", bufs=2))
        d_pool = ctx.enter_context(tc.tile_pool(name="d_in", bufs=2))
        e4t_pool = ctx.enter_context(tc.tile_pool(name="e4t", bufs=2))
        e1t_pool = ctx.enter_context(tc.tile_pool(name="e1t", bufs=2))
        e4o_pool = ctx.enter_context(tc.tile_pool(name="e4o", bufs=2))
        s2o_pool = ctx.enter_context(tc.tile_pool(name="s2o", bufs=1))
        sv_pool = ctx.enter_context(tc.tile_pool(name="scr_v", bufs=1))
        small = ctx.enter_context(tc.tile_pool(name="small", bufs=1))

        mult = mybir.AluOpType.mult
        add = mybir.AluOpType.add
        sub = mybir.AluOpType.subtract
        Exp = mybir.ActivationFunctionType.Exp
        Ln = mybir.ActivationFunctionType.Ln
        X = mybir.AxisListType.X

        # per-row-block accumulators: one column per column-tile
        QUANT = ("zt4", "zt1", "zo4", "zo1", "D", "dt1")
        acc = {}
        for rb in range(rb_count):
            for q in QUANT:
                acc[(rb, q)] = small.tile(
                    [P, nt], f32, tag=f"acc_{q}_{rb}", name=f"acc_{q}_{rb}"
                )

        otgt_sb = small.tile([P, rb_count], f32, tag="otgt", name="otgt")
        for rb in range(rb_count):
            nc.sync.dma_start(out=otgt_sb[:, rb : rb + 1], in_=otgt_ext[rb])

        def emit_rb(rb):
            r0 = rb * P
            c0 = 0
            for ci, cw in enumerate(widths):
                t_tile = t_pool.tile([P, w], bf16, tag="t_in")
                o_tile = o_pool.tile([P, w], bf16, tag="o_in")
                d_tile = d_pool.tile([P, w], bf16, tag="d_in")
                nc.sync.dma_start(out=t_tile[:, :cw], in_=tch_ext[r0 : r0 + P, c0 : c0 + cw])
                nc.sync.dma_start(out=o_tile[:, :cw], in_=outs_ext[r0 : r0 + P, c0 : c0 + cw])
                nc.sync.dma_start(out=d_tile[:, :cw], in_=diff_ext[r0 : r0 + P, c0 : c0 + cw])

                e4t = e4t_pool.tile([P, w], bf16, tag="e4t")
                e1t = e1t_pool.tile([P, w], bf16, tag="e1t")
                e4o = e4o_pool.tile([P, w], bf16, tag="e4o")

                # ScalarE: exp passes, each with a free row-sum accum
                nc.scalar.activation(
                    e4t[:, :cw], t_tile[:, :cw], Exp, scale=0.25,
                    accum_out=acc[(rb, "zt4")][:, ci : ci + 1],
                )
                nc.scalar.activation(
                    e1t[:, :cw], t_tile[:, :cw], Exp,
                    accum_out=acc[(rb, "zt1")][:, ci : ci + 1],
                )
                nc.scalar.activation(
                    e4o[:, :cw], o_tile[:, :cw], Exp, scale=0.25,
                    accum_out=acc[(rb, "zo4")][:, ci : ci + 1],
                )

                scr_v = sv_pool.tile([P, w], bf16, tag="scr_v")
                s2o = s2o_pool.tile([P, w], bf16, tag="s2o")
                if cw % 2 == 0:
                    # 2x fused product + row-sum; total extracted from the
                    # running fold in the output stream
                    dve2x.mul_total(nc, out=scr_v[:, :cw], in0=e4t[:, :cw],
                                    in1=d_tile[:, :cw],
                                    total_out=acc[(rb, "D")][:, ci : ci + 1], cw=cw)
                    dve2x.mul_total(nc, out=scr_v[:, :cw], in0=e1t[:, :cw],
                                    in1=t_tile[:, :cw],
                                    total_out=acc[(rb, "dt1")][:, ci : ci + 1], cw=cw)
                    nc.vector.tensor_tensor(
                        out=s2o[:, :cw], in0=e4o[:, :cw], in1=e4o[:, :cw], op=mult
                    )
                    dve2x.mul_total(nc, out=scr_v[:, :cw], in0=s2o[:, :cw],
                                    in1=s2o[:, :cw],
                                    total_out=acc[(rb, "zo1")][:, ci : ci + 1], cw=cw)
                else:
                    # odd tail: 1x hardware-accumulator path
                    dve2x.mul_acc(nc, out=scr_v[:, :cw], in0=e4t[:, :cw],
                                  in1=d_tile[:, :cw],
                                  accum_out=acc[(rb, "D")][:, ci : ci + 1])
                    dve2x.mul_acc(nc, out=scr_v[:, :cw], in0=e1t[:, :cw],
                                  in1=t_tile[:, :cw],
                                  accum_out=acc[(rb, "dt1")][:, ci : ci + 1])
                    nc.vector.tensor_tensor(
                        out=s2o[:, :cw], in0=e4o[:, :cw], in1=e4o[:, :cw], op=mult
                    )
                    dve2x.mul_acc(nc, out=scr_v[:, :cw], in0=s2o[:, :cw],
                                  in1=s2o[:, :cw],
                                  accum_out=acc[(rb, "zo1")][:, ci : ci + 1])
                c0 += cw

        def emit_epilogue(rb):
            # collapse per-tile partials: res columns follow QUANT order
            res = small.tile([P, 6], f32, tag=f"res_{rb}", name=f"res_{rb}")
            for qi, q in enumerate(QUANT):
                nc.vector.tensor_reduce(
                    out=res[:, qi : qi + 1], in_=acc[(rb, q)][:, :nt], axis=X, op=add
                )

            # lse = [log zt4, log zt1, log zo4, log zo1]
            lse = small.tile([P, 4], f32, tag=f"lse_{rb}", name=f"lse_{rb}")
            nc.scalar.activation(lse[:, :4], res[:, 0:4], Ln)
            # reciprocals of zt4, zt1
            rcp = small.tile([P, 2], f32, tag=f"rcp_{rb}", name=f"rcp_{rb}")
            nc.vector.reciprocal(out=rcp[:, :2], in_=res[:, 0:2])

            tmp = small.tile([P, 4], f32, tag=f"tmp_{rb}", name=f"tmp_{rb}")
            # tmp0 = H = log(zt1) - dt1/zt1
            nc.vector.tensor_tensor(tmp[:, 0:1], res[:, 5:6], rcp[:, 1:2], op=mult)
            nc.vector.tensor_tensor(tmp[:, 0:1], lse[:, 1:2], tmp[:, 0:1], op=sub)
            # tmp0 = alpha = clip(1 - H/lnC, 0, 1)
            nc.vector.tensor_scalar(
                tmp[:, 0:1], tmp[:, 0:1], -1.0 / ln_c, 1.0, op0=mult, op1=add
            )
            nc.vector.tensor_scalar(
                tmp[:, 0:1], tmp[:, 0:1], 0.0, 1.0,
                op0=mybir.AluOpType.max, op1=mybir.AluOpType.min,
            )
            # tmp1 = ce = log(zo1) - o[tgt]
            nc.vector.tensor_tensor(tmp[:, 1:2], lse[:, 3:4], otgt_sb[:, rb : rb + 1], op=sub)
            # tmp2 = kl = D*0.25/zt4 + (log zo4 - log zt4)
            nc.vector.tensor_tensor(tmp[:, 2:3], res[:, 4:5], rcp[:, 0:1], op=mult)
            nc.vector.tensor_scalar(tmp[:, 2:3], tmp[:, 2:3], 0.25, None, op0=mult)
            nc.vector.tensor_tensor(tmp[:, 3:4], lse[:, 2:3], lse[:, 0:1], op=sub)
            nc.vector.tensor_tensor(tmp[:, 2:3], tmp[:, 2:3], tmp[:, 3:4], op=add)
            # loss = ce + alpha*(16*kl - ce)
            nc.vector.tensor_scalar(tmp[:, 2:3], tmp[:, 2:3], 16.0, None, op0=mult)
            nc.vector.tensor_tensor(tmp[:, 2:3], tmp[:, 2:3], tmp[:, 1:2], op=sub)
            loss_sb = small.tile([P, 1], f32, tag=f"loss_{rb}", name=f"loss_{rb}")
            nc.vector.tensor_tensor(loss_sb[:, :], tmp[:, 0:1], tmp[:, 2:3], op=mult)
            nc.vector.tensor_tensor(loss_sb[:, :], loss_sb[:, :], tmp[:, 1:2], op=add)
            nc.sync.dma_start(out=loss_ext[rb], in_=loss_sb[:, :])

        for rb in range(rb_count):
            emit_rb(rb)
        # epilogues after all exp streaming: a single Exp->Ln table switch
        for rb in range(rb_count):
            emit_epilogue(rb)

    nc.compile()
    dve2x.enable_2x_on_module(nc)
    return nc


def make_in_maps(outputs, teacher_outputs, targets):
    outputs = np.ascontiguousarray(outputs, dtype=np.float32)
    teacher = np.ascontiguousarray(teacher_outputs, dtype=np.float32)
    tgt = np.asarray(targets).astype(np.int64).reshape(-1)
    t16 = teacher.astype(BF16)
    o16 = outputs.astype(BF16)
    d16 = (teacher - outputs).astype(BF16)
    otgt = outputs[np.arange(B), tgt].astype(np.float32)
    in_maps = []
    for i in range(N_CORES):
        r0 = i * RPC
        in_maps.append(
            {
                "teacher": t16[r0 : r0 + RPC],
                "outputs": o16[r0 : r0 + RPC],
                "diff": d16[r0 : r0 + RPC],
                "otgt": otgt[r0 : r0 + RPC].reshape(RB, P, 1),
            }
        )
    return in_maps


_NC_CACHE = {}


def _get_nc():
    if "nc" not in _NC_CACHE:
        _NC_CACHE["nc"] = build_nc()
    return _NC_CACHE["nc"]


def run(outputs, teacher_outputs, targets, trace=False, tmpdir=None):
    """Run on hardware; returns (per_sample[2048], BassKernelResults)."""
    from concourse.bass_utils import run_bass_kernel_spmd

    nc = _get_nc()
    in_maps = make_in_maps(outputs, teacher_outputs, targets)
    res = run_bass_kernel_spmd(
        nc, in_maps, core_ids=list(range(N_CORES)), trace=trace, tmpdir=tmpdir
    )
    per_sample = np.concatenate([r["loss"].reshape(-1) for r in res.results])
    return per_sample, res


def kernel(outputs, teacher_outputs, targets):
    per_sample, _ = run(outputs, teacher_outputs, targets)
    return np.float32(per_sample.mean(dtype=np.float64))


# revision 7
# speedup vs baseline: 1.1383x; 1.1383x over previous
"""Adaptive weighted knowledge-distillation loss on 8 TRN2 NeuronCores.

Pure data parallel: the batch (2048 rows) is split into 8 shards of 256
rows; each core streams its [256, 50257] shard and computes per-row
reductions over the class axis; the host averages the gathered [2048]
per-sample losses.

Inputs are uploaded as bf16 (tolerance is 2e-2; bf16 end-to-end error is
~2e-5), which halves HBM traffic. A third bf16 tensor d = t - o is
prepared on the host because the KL cross term only needs
D = sum(exp(t/4) * (t - o)); this removes one full fused product pass.
The per-row o[target] values are gathered on the host (f32, exact) and
uploaded, replacing the indirect-DMA gather.

Per-core math (row t = teacher logits, o = student logits, T = 4):
    zt4 = sum e^{t/4}   zt1 = sum e^t     zo4 = sum e^{o/4}  zo1 = sum e^o
    D   = sum e^{t/4} (t-o)               dt1 = sum t e^t
    H     = log zt1 - dt1/zt1
    alpha = clip(1 - H/log C, 0, 1)
    ce    = log zo1 - o[tgt]
    kl    = D/(4 zt4) - log zt4 + log zo4
    loss  = (1-alpha) ce + 16 alpha kl
No max-subtraction is needed: logits are standard-normal, exp() stays
comfortably inside f32/bf16 range.

Engine budget (measured rates, per core): ScalarE activation runs 1
elem/cycle/lane at any dtype (83.8us per full pass); plain
tensor_tensor bf16 runs 2x on DVE (52.4us); the stock fused
product+row-sum ops only run 1x (104.7us), so dve2x.py registers a
custom DVE op with a hand-authored 2X_1PORT uop program whose running
fold lands in the last even output element (mul_total, 52.4us/pass).
Work split:
  ScalarE (3 passes): e4t (zt4 accum), e1t (zt1 accum), e4o (zo4 accum)
  VectorE: mul_total(e4t, d) -> D, mul_total(e1t, t) -> dt1,
           s2o = e4o*e4o then mul_total(s2o, s2o) -> zo1 = sum e^o
The odd-width tail tile uses the 1x hardware-accumulator path
(mul_acc); every other width is even so the 2x program engages.
"""

import sys

import numpy as np

try:
    import concourse  # noqa: F401
except ImportError:  # platform checkout location in the bench containers
    sys.path.insert(0, "/opt/trn_rl_repo")

import ml_dtypes

BF16 = ml_dtypes.bfloat16

B, C = 2048, 50257
N_CORES = 8
RPC = B // N_CORES  # rows per core = 256
P = 128  # SBUF partitions
RB = RPC // P  # row blocks per core = 2
W = 5632  # column tile width
LN_C = float(np.log(np.float32(C)))


def build_nc(rows=RPC, n_classes=C, w=W, debug=False):
    """Build the per-core Tile kernel (same SPMD graph for all cores)."""
    from contextlib import ExitStack

    import concourse.bacc as bacc
    import concourse.tile as tile
    from concourse import mybir

    import dve2x

    f32 = mybir.dt.float32
    bf16 = mybir.dt.bfloat16
    rb_count = rows // P
    assert rows % P == 0
    ln_c = float(np.log(np.float32(n_classes)))

    nc = bacc.Bacc("TRN2", target_bir_lowering=False, debug=debug)

    tch_ext = nc.declare_dram_parameter("teacher", [rows, n_classes], bf16, isOutput=False)
    outs_ext = nc.declare_dram_parameter("outputs", [rows, n_classes], bf16, isOutput=False)
    diff_ext = nc.declare_dram_parameter("diff", [rows, n_classes], bf16, isOutput=False)
    otgt_ext = nc.declare_dram_parameter("otgt", [rb_count, P, 1], f32, isOutput=False)
    loss_ext = nc.declare_dram_parameter("loss", [rb_count, P, 1], f32, isOutput=True)

    # Column tile schedule: first tile split small (odd - it takes the 1x
    # path and doubles as the pipeline warm-up), every other tile even so
    # the 2x DVE program engages; sums to n_classes exactly.
    n_full = n_classes // w - 1
    head = n_classes - n_full * w
    h1 = w // 4 + 1  # odd
    h2 = head - h1
    widths = [h1, (h2 // 2) & ~1, h2 - ((h2 // 2) & ~1)] + [w] * n_full
    assert sum(widths) == n_classes
    assert all(x % 2 == 0 for x in widths[1:]) and all(x <= w for x in widths)
    nt = len(widths)
    # tiles where e1t/zt1 move from ScalarE to the VectorE squaring chain
    # (e2t = e4t^2, e1t = e2t^2, zt1 = sum e2t^2) to balance the engines
    x_tiles = {3, 6}

    with tile.TileContext(nc) as tc, ExitStack() as ctx:
        t_pool = ctx.enter_context(tc.tile_pool(name="t_in", bufs=3))
        o_pool = ctx.enter_context(tc.tile_pool(name="o_in", bufs=3))
        d_pool = ctx.enter_context(tc.tile_pool(name="d_in", bufs=2))
        e4t_pool = ctx.enter_context(tc.tile_pool(name="e4t", bufs=2))
        e1t_pool = ctx.enter_context(tc.tile_pool(name="e1t", bufs=2))
        e4o_pool = ctx.enter_context(tc.tile_pool(name="e4o", bufs=2))
        s2o_pool = ctx.enter_context(tc.tile_pool(name="s2o", bufs=1))
        sv_pool = ctx.enter_context(tc.tile_pool(name="scr_v", bufs=1))
        small = ctx.enter_context(tc.tile_pool(name="small", bufs=1))

        mult = mybir.AluOpType.mult
        add = mybir.AluOpType.add
        sub = mybir.AluOpType.subtract
        Exp = mybir.ActivationFunctionType.Exp
        Ln = mybir.ActivationFunctionType.Ln
        X = mybir.AxisListType.X

        # per-row-block accumulators: one column per column-tile
        QUANT = ("zt4", "zt1", "zo4", "zo1", "D", "dt1")
        acc = {}
        for rb in range(rb_count):
            for q in QUANT:
                acc[(rb, q)] = small.tile(
                    [P, nt], f32, tag=f"acc_{q}_{rb}", name=f"acc_{q}_{rb}"
                )

        otgt_sb = small.tile([P, rb_count], f32, tag="otgt", name="otgt")
        for rb in range(rb_count):
            nc.sync.dma_start(out=otgt_sb[:, rb : rb + 1], in_=otgt_ext[rb])

        def emit_rb(rb):
            r0 = rb * P
            c0 = 0
            for ci, cw in enumerate(widths):
                t_tile = t_pool.tile([P, w], bf16, tag="t_in")
                o_tile = o_pool.tile([P, w], bf16, tag="o_in")
                d_tile = d_pool.tile([P, w], bf16, tag="d_in")
                nc.sync.dma_start(out=t_tile[:, :cw], in_=tch_ext[r0 : r0 + P, c0 : c0 + cw])
                nc.sync.dma_start(out=o_tile[:, :cw], in_=outs_ext[r0 : r0 + P, c0 : c0 + cw])
                nc.sync.dma_start(out=d_tile[:, :cw], in_=diff_ext[r0 : r0 + P, c0 : c0 + cw])

                e4t = e4t_pool.tile([P, w], bf16, tag="e4t")
                e1t = e1t_pool.tile([P, w], bf16, tag="e1t")
                e4o = e4o_pool.tile([P, w], bf16, tag="e4o")

                # ScalarE: exp passes, each with a free row-sum accum
                nc.scalar.activation(
                    e4t[:, :cw], t_tile[:, :cw], Exp, scale=0.25,
                    accum_out=acc[(rb, "zt4")][:, ci : ci + 1],
                )
                scr_v = sv_pool.tile([P, w], bf16, tag="scr_v")
                s2o = s2o_pool.tile([P, w], bf16, tag="s2o")
                if ci in x_tiles and cw % 2 == 0:
                    # VectorE chain: e2t = (e^{t/4})^2 (borrows the s2o
                    # tile before s2o is computed), e1t = e2t^2,
                    # zt1 = sum e2t^2
                    nc.vector.tensor_tensor(
                        out=s2o[:, :cw], in0=e4t[:, :cw], in1=e4t[:, :cw], op=mult
                    )
                    nc.vector.tensor_tensor(
                        out=e1t[:, :cw], in0=s2o[:, :cw], in1=s2o[:, :cw], op=mult
                    )
                    dve2x.mul_total(nc, out=scr_v[:, :cw], in0=s2o[:, :cw],
                                    in1=s2o[:, :cw],
                                    total_out=acc[(rb, "zt1")][:, ci : ci + 1], cw=cw)
                else:
                    nc.scalar.activation(
                        e1t[:, :cw], t_tile[:, :cw], Exp,
                        accum_out=acc[(rb, "zt1")][:, ci : ci + 1],
                    )
                nc.scalar.activation(
                    e4o[:, :cw], o_tile[:, :cw], Exp, scale=0.25,
                    accum_out=acc[(rb, "zo4")][:, ci : ci + 1],
                )

                if cw % 2 == 0:
                    # 2x fused product + row-sum; total extracted from the
                    # running fold in the output stream
                    dve2x.mul_total(nc, out=scr_v[:, :cw], in0=e4t[:, :cw],
                                    in1=d_tile[:, :cw],
                                    total_out=acc[(rb, "D")][:, ci : ci + 1], cw=cw)
                    dve2x.mul_total(nc, out=scr_v[:, :cw], in0=e1t[:, :cw],
                                    in1=t_tile[:, :cw],
                                    total_out=acc[(rb, "dt1")][:, ci : ci + 1], cw=cw)
                    nc.vector.tensor_tensor(
                        out=s2o[:, :cw], in0=e4o[:, :cw], in1=e4o[:, :cw], op=mult
                    )
                    dve2x.mul_total(nc, out=scr_v[:, :cw], in0=s2o[:, :cw],
                                    in1=s2o[:, :cw],
                                    total_out=acc[(rb, "zo1")][:, ci : ci + 1], cw=cw)
                else:
                    # odd tail: 1x hardware-accumulator path
                    dve2x.mul_acc(nc, out=scr_v[:, :cw], in0=e4t[:, :cw],
                                  in1=d_tile[:, :cw],
                                  accum_out=acc[(rb, "D")][:, ci : ci + 1])
                    dve2x.mul_acc(nc, out=scr_v[:, :cw], in0=e1t[:, :cw],
                                  in1=t_tile[:, :cw],
                                  accum_out=acc[(rb, "dt1")][:, ci : ci + 1])
                    nc.vector.tensor_tensor(
                        out=s2o[:, :cw], in0=e4o[:, :cw], in1=e4o[:, :cw], op=mult
                    )
                    dve2x.mul_acc(nc, out=scr_v[:, :cw], in0=s2o[:, :cw],
                                  in1=s2o[:, :cw],
                                  accum_out=acc[(rb, "zo1")][:, ci : ci + 1])
                c0 += cw

        def emit_epilogue():
            # collapse per-tile partials; column r of each res tile = row
            # block r, so the whole scalar tail is one short op chain
            nrb = rb_count
            res = {}
            for q in QUANT:
                res[q] = small.tile([P, nrb], f32, tag=f"res_{q}", name=f"res_{q}")
                for rb in range(nrb):
                    nc.vector.tensor_reduce(
                        out=res[q][:, rb : rb + 1], in_=acc[(rb, q)][:, :nt],
                        axis=X, op=add,
                    )
            # lse tile: [zt4 | zt1 | zo4 | zo1] x rb  (one Ln instruction)
            zcat = small.tile([P, 4 * nrb], f32, tag="zcat", name="zcat")
            for qi, q in enumerate(("zt4", "zt1", "zo4", "zo1")):
                nc.vector.tensor_copy(
                    out=zcat[:, qi * nrb : (qi + 1) * nrb], in_=res[q][:, :]
                )
            lse = small.tile([P, 4 * nrb], f32, tag="lse", name="lse")
            nc.scalar.activation(lse[:, :], zcat[:, :], Ln)
            l_zt4 = lse[:, 0 * nrb : 1 * nrb]
            l_zt1 = lse[:, 1 * nrb : 2 * nrb]
            l_zo4 = lse[:, 2 * nrb : 3 * nrb]
            l_zo1 = lse[:, 3 * nrb : 4 * nrb]
            rcp = small.tile([P, 2 * nrb], f32, tag="rcp", name="rcp")
            nc.vector.reciprocal(out=rcp[:, : 2 * nrb], in_=zcat[:, : 2 * nrb])
            r_zt4 = rcp[:, 0 * nrb : 1 * nrb]
            r_zt1 = rcp[:, 1 * nrb : 2 * nrb]

            tmp = small.tile([P, 4 * nrb], f32, tag="tmp", name="tmp")
            a_ = tmp[:, 0 * nrb : 1 * nrb]
            ce = tmp[:, 1 * nrb : 2 * nrb]
            kl = tmp[:, 2 * nrb : 3 * nrb]
            t3 = tmp[:, 3 * nrb : 4 * nrb]
            # alpha = clip(1 - (log zt1 - dt1/zt1)/lnC, 0, 1)
            nc.vector.tensor_tensor(a_, res["dt1"][:, :], r_zt1, op=mult)
            nc.vector.tensor_tensor(a_, l_zt1, a_, op=sub)
            nc.vector.tensor_scalar(a_, a_, -1.0 / ln_c, 1.0, op0=mult, op1=add)
            nc.vector.tensor_scalar(
                a_, a_, 0.0, 1.0,
                op0=mybir.AluOpType.max, op1=mybir.AluOpType.min,
            )
            # ce = log(zo1) - o[tgt]
            nc.vector.tensor_tensor(ce, l_zo1, otgt_sb[:, :], op=sub)
            # kl = D*0.25/zt4 + (log zo4 - log zt4)
            nc.vector.tensor_tensor(kl, res["D"][:, :], r_zt4, op=mult)
            nc.vector.tensor_scalar(kl, kl, 0.25, None, op0=mult)
            nc.vector.tensor_tensor(t3, l_zo4, l_zt4, op=sub)
            nc.vector.tensor_tensor(kl, kl, t3, op=add)
            # loss = ce + alpha*(16*kl - ce)
            nc.vector.tensor_scalar(kl, kl, 16.0, None, op0=mult)
            nc.vector.tensor_tensor(kl, kl, ce, op=sub)
            loss_sb = small.tile([P, nrb], f32, tag="loss", name="loss")
            nc.vector.tensor_tensor(loss_sb[:, :], a_, kl, op=mult)
            nc.vector.tensor_tensor(loss_sb[:, :], loss_sb[:, :], ce, op=add)
            for rb in range(nrb):
                nc.sync.dma_start(out=loss_ext[rb], in_=loss_sb[:, rb : rb + 1])

        for rb in range(rb_count):
            emit_rb(rb)
        emit_epilogue()

    nc.compile()
    dve2x.enable_2x_on_module(nc)
    return nc


def make_in_maps(outputs, teacher_outputs, targets):
    outputs = np.ascontiguousarray(outputs, dtype=np.float32)
    teacher = np.ascontiguousarray(teacher_outputs, dtype=np.float32)
    tgt = np.asarray(targets).astype(np.int64).reshape(-1)
    t16 = teacher.astype(BF16)
    o16 = outputs.astype(BF16)
    d16 = (teacher - outputs).astype(BF16)
    otgt = outputs[np.arange(B), tgt].astype(np.float32)
    in_maps = []
    for i in range(N_CORES):
        r0 = i * RPC
        in_maps.append(
            {
                "teacher": t16[r0 : r0 + RPC],
                "outputs": o16[r0 : r0 + RPC],
                "diff": d16[r0 : r0 + RPC],
                "otgt": otgt[r0 : r0 + RPC].reshape(RB, P, 1),
            }
        )
    return in_maps


_NC_CACHE = {}


def _get_nc():
    if "nc" not in _NC_CACHE:
        _NC_CACHE["nc"] = build_nc()
    return _NC_CACHE["nc"]


def run(outputs, teacher_outputs, targets, trace=False, tmpdir=None):
    """Run on hardware; returns (per_sample[2048], BassKernelResults)."""
    from concourse.bass_utils import run_bass_kernel_spmd

    nc = _get_nc()
    in_maps = make_in_maps(outputs, teacher_outputs, targets)
    res = run_bass_kernel_spmd(
        nc, in_maps, core_ids=list(range(N_CORES)), trace=trace, tmpdir=tmpdir
    )
    per_sample = np.concatenate([r["loss"].reshape(-1) for r in res.results])
    return per_sample, res


def kernel(outputs, teacher_outputs, targets):
    per_sample, _ = run(outputs, teacher_outputs, targets)
    return np.float32(per_sample.mean(dtype=np.float64))


# revision 8
# speedup vs baseline: 1.1674x; 1.0256x over previous
"""Adaptive weighted knowledge-distillation loss on 8 TRN2 NeuronCores.

Pure data parallel: the batch (2048 rows) is split into 8 shards of 256
rows; each core streams its [256, 50257] shard and computes per-row
reductions over the class axis; the host averages the gathered [2048]
per-sample losses.

Inputs are uploaded as bf16 (tolerance is 2e-2; bf16 end-to-end error is
~2e-5), which halves HBM traffic. A third bf16 tensor d = t - o is
prepared on the host because the KL cross term only needs
D = sum(exp(t/4) * (t - o)); this removes one full fused product pass.
The per-row o[target] values are gathered on the host (f32, exact) and
uploaded, replacing the indirect-DMA gather.

Per-core math (row t = teacher logits, o = student logits, T = 4):
    zt4 = sum e^{t/4}   zt1 = sum e^t     zo4 = sum e^{o/4}  zo1 = sum e^o
    D   = sum e^{t/4} (t-o)               dt1 = sum t e^t
    H     = log zt1 - dt1/zt1
    alpha = clip(1 - H/log C, 0, 1)
    ce    = log zo1 - o[tgt]
    kl    = D/(4 zt4) - log zt4 + log zo4
    loss  = (1-alpha) ce + 16 alpha kl
No max-subtraction is needed: logits are standard-normal, exp() stays
comfortably inside f32/bf16 range.

Engine budget (measured rates, per core): ScalarE activation runs 1
elem/cycle/lane at any dtype (83.8us per full pass); plain
tensor_tensor bf16 runs 2x on DVE (52.4us); the stock fused
product+row-sum ops only run 1x (104.7us), so dve2x.py registers a
custom DVE op with a hand-authored 2X_1PORT uop program whose running
fold lands in the last even output element (mul_total, 52.4us/pass).
Work split:
  ScalarE (3 passes): e4t (zt4 accum), e1t (zt1 accum), e4o (zo4 accum)
  VectorE: mul_total(e4t, d) -> D, mul_total(e1t, t) -> dt1,
           s2o = e4o*e4o then mul_total(s2o, s2o) -> zo1 = sum e^o
The odd-width tail tile uses the 1x hardware-accumulator path
(mul_acc); every other width is even so the 2x program engages.
"""

import sys

import numpy as np

try:
    import concourse  # noqa: F401
except ImportError:  # platform checkout location in the bench containers
    sys.path.insert(0, "/opt/trn_rl_repo")

import ml_dtypes

BF16 = ml_dtypes.bfloat16

B, C = 2048, 50257
N_CORES = 8
RPC = B // N_CORES  # rows per core = 256
P = 128  # SBUF partitions
RB = RPC // P  # row blocks per core = 2
W = 5632  # column tile width
LN_C = float(np.log(np.float32(C)))


def build_nc(rows=RPC, n_classes=C, w=W, debug=False):
    """Build the per-core Tile kernel (same SPMD graph for all cores)."""
    from contextlib import ExitStack

    import concourse.bacc as bacc
    import concourse.tile as tile
    from concourse import mybir

    import dve2x

    f32 = mybir.dt.float32
    bf16 = mybir.dt.bfloat16
    rb_count = rows // P
    assert rows % P == 0
    ln_c = float(np.log(np.float32(n_classes)))

    nc = bacc.Bacc("TRN2", target_bir_lowering=False, debug=debug)

    tch_ext = nc.declare_dram_parameter("teacher", [rows, n_classes], bf16, isOutput=False)
    outs_ext = nc.declare_dram_parameter("outputs", [rows, n_classes], bf16, isOutput=False)
    diff_ext = nc.declare_dram_parameter("diff", [rows, n_classes], bf16, isOutput=False)
    otgt_ext = nc.declare_dram_parameter("otgt", [rb_count, P, 1], f32, isOutput=False)
    loss_ext = nc.declare_dram_parameter("loss", [rb_count, P, 1], f32, isOutput=True)

    # Column tile schedule: first tile split small (odd - it takes the 1x
    # path and doubles as the pipeline warm-up), every other tile even so
    # the 2x DVE program engages; sums to n_classes exactly.
    n_full = n_classes // w - 1
    head = n_classes - n_full * w
    h1 = w // 4 + 1  # odd, 1x path, doubles as pipeline warm-up
    h2 = head - h1
    q1 = (h2 // 2) & ~1
    # small even tiles at the end so the engine pipeline drains quickly
    widths = [h1, q1, h2 - q1] + [w] * (n_full - 1) + [w // 2, w - w // 2]
    assert sum(widths) == n_classes
    assert all(x % 2 == 0 for x in widths[1:]) and all(x <= w for x in widths)
    nt = len(widths)
    # tiles where e1t/zt1 move from ScalarE to the VectorE squaring chain
    # (e2t = e4t^2, e1t = e2t^2, zt1 = sum e2t^2) to balance the engines
    x_tiles = {3, 6}

    with tile.TileContext(nc) as tc, ExitStack() as ctx:
        t_pool = ctx.enter_context(tc.tile_pool(name="t_in", bufs=3))
        o_pool = ctx.enter_context(tc.tile_pool(name="o_in", bufs=3))
        d_pool = ctx.enter_context(tc.tile_pool(name="d_in", bufs=2))
        e4t_pool = ctx.enter_context(tc.tile_pool(name="e4t", bufs=2))
        e1t_pool = ctx.enter_context(tc.tile_pool(name="e1t", bufs=2))
        e4o_pool = ctx.enter_context(tc.tile_pool(name="e4o", bufs=2))
        s2o_pool = ctx.enter_context(tc.tile_pool(name="s2o", bufs=1))
        sv_pool = ctx.enter_context(tc.tile_pool(name="scr_v", bufs=1))
        small = ctx.enter_context(tc.tile_pool(name="small", bufs=1))

        mult = mybir.AluOpType.mult
        add = mybir.AluOpType.add
        sub = mybir.AluOpType.subtract
        Exp = mybir.ActivationFunctionType.Exp
        Ln = mybir.ActivationFunctionType.Ln
        X = mybir.AxisListType.X

        # per-row-block accumulators: one column per column-tile
        QUANT = ("zt4", "zt1", "zo4", "zo1", "D", "dt1")
        acc = {}
        for rb in range(rb_count):
            for q in QUANT:
                acc[(rb, q)] = small.tile(
                    [P, nt], f32, tag=f"acc_{q}_{rb}", name=f"acc_{q}_{rb}"
                )

        otgt_sb = small.tile([P, rb_count], f32, tag="otgt", name="otgt")
        for rb in range(rb_count):
            nc.sync.dma_start(out=otgt_sb[:, rb : rb + 1], in_=otgt_ext[rb])

        def emit_rb(rb):
            r0 = rb * P
            c0 = 0
            for ci, cw in enumerate(widths):
                t_tile = t_pool.tile([P, w], bf16, tag="t_in")
                o_tile = o_pool.tile([P, w], bf16, tag="o_in")
                d_tile = d_pool.tile([P, w], bf16, tag="d_in")
                nc.sync.dma_start(out=t_tile[:, :cw], in_=tch_ext[r0 : r0 + P, c0 : c0 + cw])
                nc.sync.dma_start(out=o_tile[:, :cw], in_=outs_ext[r0 : r0 + P, c0 : c0 + cw])
                nc.sync.dma_start(out=d_tile[:, :cw], in_=diff_ext[r0 : r0 + P, c0 : c0 + cw])

                e4t = e4t_pool.tile([P, w], bf16, tag="e4t")
                e1t = e1t_pool.tile([P, w], bf16, tag="e1t")
                e4o = e4o_pool.tile([P, w], bf16, tag="e4o")

                # ScalarE: exp passes, each with a free row-sum accum
                nc.scalar.activation(
                    e4t[:, :cw], t_tile[:, :cw], Exp, scale=0.25,
                    accum_out=acc[(rb, "zt4")][:, ci : ci + 1],
                )
                scr_v = sv_pool.tile([P, w], bf16, tag="scr_v")
                s2o = s2o_pool.tile([P, w], bf16, tag="s2o")
                if ci in x_tiles and cw % 2 == 0:
                    # VectorE chain: e2t = (e^{t/4})^2 (borrows the s2o
                    # tile before s2o is computed), e1t = e2t^2,
                    # zt1 = sum e2t^2
                    nc.vector.tensor_tensor(
                        out=s2o[:, :cw], in0=e4t[:, :cw], in1=e4t[:, :cw], op=mult
                    )
                    nc.vector.tensor_tensor(
                        out=e1t[:, :cw], in0=s2o[:, :cw], in1=s2o[:, :cw], op=mult
                    )
                    dve2x.mul_total(nc, out=scr_v[:, :cw], in0=s2o[:, :cw],
                                    in1=s2o[:, :cw],
                                    total_out=acc[(rb, "zt1")][:, ci : ci + 1], cw=cw)
                else:
                    nc.scalar.activation(
                        e1t[:, :cw], t_tile[:, :cw], Exp,
                        accum_out=acc[(rb, "zt1")][:, ci : ci + 1],
                    )
                nc.scalar.activation(
                    e4o[:, :cw], o_tile[:, :cw], Exp, scale=0.25,
                    accum_out=acc[(rb, "zo4")][:, ci : ci + 1],
                )

                if cw % 2 == 0:
                    # 2x fused product + row-sum; total extracted from the
                    # running fold in the output stream
                    dve2x.mul_total(nc, out=scr_v[:, :cw], in0=e4t[:, :cw],
                                    in1=d_tile[:, :cw],
                                    total_out=acc[(rb, "D")][:, ci : ci + 1], cw=cw)
                    dve2x.mul_total(nc, out=scr_v[:, :cw], in0=e1t[:, :cw],
                                    in1=t_tile[:, :cw],
                                    total_out=acc[(rb, "dt1")][:, ci : ci + 1], cw=cw)
                    nc.vector.tensor_tensor(
                        out=s2o[:, :cw], in0=e4o[:, :cw], in1=e4o[:, :cw], op=mult
                    )
                    dve2x.mul_total(nc, out=scr_v[:, :cw], in0=s2o[:, :cw],
                                    in1=s2o[:, :cw],
                                    total_out=acc[(rb, "zo1")][:, ci : ci + 1], cw=cw)
                else:
                    # odd tail: 1x hardware-accumulator path
                    dve2x.mul_acc(nc, out=scr_v[:, :cw], in0=e4t[:, :cw],
                                  in1=d_tile[:, :cw],
                                  accum_out=acc[(rb, "D")][:, ci : ci + 1])
                    dve2x.mul_acc(nc, out=scr_v[:, :cw], in0=e1t[:, :cw],
                                  in1=t_tile[:, :cw],
                                  accum_out=acc[(rb, "dt1")][:, ci : ci + 1])
                    nc.vector.tensor_tensor(
                        out=s2o[:, :cw], in0=e4o[:, :cw], in1=e4o[:, :cw], op=mult
                    )
                    dve2x.mul_acc(nc, out=scr_v[:, :cw], in0=s2o[:, :cw],
                                  in1=s2o[:, :cw],
                                  accum_out=acc[(rb, "zo1")][:, ci : ci + 1])
                c0 += cw

        def emit_epilogue():
            # collapse per-tile partials; column r of each res tile = row
            # block r, so the whole scalar tail is one short op chain
            nrb = rb_count
            res = {}
            for q in QUANT:
                res[q] = small.tile([P, nrb], f32, tag=f"res_{q}", name=f"res_{q}")
                for rb in range(nrb):
                    nc.vector.tensor_reduce(
                        out=res[q][:, rb : rb + 1], in_=acc[(rb, q)][:, :nt],
                        axis=X, op=add,
                    )
            # lse tile: [zt4 | zt1 | zo4 | zo1] x rb  (one Ln instruction)
            zcat = small.tile([P, 4 * nrb], f32, tag="zcat", name="zcat")
            for qi, q in enumerate(("zt4", "zt1", "zo4", "zo1")):
                nc.vector.tensor_copy(
                    out=zcat[:, qi * nrb : (qi + 1) * nrb], in_=res[q][:, :]
                )
            lse = small.tile([P, 4 * nrb], f32, tag="lse", name="lse")
            nc.scalar.activation(lse[:, :], zcat[:, :], Ln)
            l_zt4 = lse[:, 0 * nrb : 1 * nrb]
            l_zt1 = lse[:, 1 * nrb : 2 * nrb]
            l_zo4 = lse[:, 2 * nrb : 3 * nrb]
            l_zo1 = lse[:, 3 * nrb : 4 * nrb]
            rcp = small.tile([P, 2 * nrb], f32, tag="rcp", name="rcp")
            nc.vector.reciprocal(out=rcp[:, : 2 * nrb], in_=zcat[:, : 2 * nrb])
            r_zt4 = rcp[:, 0 * nrb : 1 * nrb]
            r_zt1 = rcp[:, 1 * nrb : 2 * nrb]

            tmp = small.tile([P, 4 * nrb], f32, tag="tmp", name="tmp")
            a_ = tmp[:, 0 * nrb : 1 * nrb]
            ce = tmp[:, 1 * nrb : 2 * nrb]
            kl = tmp[:, 2 * nrb : 3 * nrb]
            t3 = tmp[:, 3 * nrb : 4 * nrb]
            # alpha = clip(1 - (log zt1 - dt1/zt1)/lnC, 0, 1)
            nc.vector.tensor_tensor(a_, res["dt1"][:, :], r_zt1, op=mult)
            nc.vector.tensor_tensor(a_, l_zt1, a_, op=sub)
            nc.vector.tensor_scalar(a_, a_, -1.0 / ln_c, 1.0, op0=mult, op1=add)
            nc.vector.tensor_scalar(
                a_, a_, 0.0, 1.0,
                op0=mybir.AluOpType.max, op1=mybir.AluOpType.min,
            )
            # ce = log(zo1) - o[tgt]
            nc.vector.tensor_tensor(ce, l_zo1, otgt_sb[:, :], op=sub)
            # kl = D*0.25/zt4 + (log zo4 - log zt4)
            nc.vector.tensor_tensor(kl, res["D"][:, :], r_zt4, op=mult)
            nc.vector.tensor_scalar(kl, kl, 0.25, None, op0=mult)
            nc.vector.tensor_tensor(t3, l_zo4, l_zt4, op=sub)
            nc.vector.tensor_tensor(kl, kl, t3, op=add)
            # loss = ce + alpha*(16*kl - ce)
            nc.vector.tensor_scalar(kl, kl, 16.0, None, op0=mult)
            nc.vector.tensor_tensor(kl, kl, ce, op=sub)
            loss_sb = small.tile([P, nrb], f32, tag="loss", name="loss")
            nc.vector.tensor_tensor(loss_sb[:, :], a_, kl, op=mult)
            nc.vector.tensor_tensor(loss_sb[:, :], loss_sb[:, :], ce, op=add)
            for rb in range(nrb):
                nc.sync.dma_start(out=loss_ext[rb], in_=loss_sb[:, rb : rb + 1])

        for rb in range(rb_count):
            emit_rb(rb)
        emit_epilogue()

    nc.compile()
    dve2x.enable_2x_on_module(nc)
    return nc


def make_in_maps(outputs, teacher_outputs, targets):
    outputs = np.ascontiguousarray(outputs, dtype=np.float32)
    teacher = np.ascontiguousarray(teacher_outputs, dtype=np.float32)
    tgt = np.asarray(targets).astype(np.int64).reshape(-1)
    t16 = teacher.astype(BF16)
    o16 = outputs.astype(BF16)
    d16 = (teacher - outputs).astype(BF16)
    otgt = outputs[np.arange(B), tgt].astype(np.float32)
    in_maps = []
    for i in range(N_CORES):
        r0 = i * RPC
        in_maps.append(
            {
                "teacher": t16[r0 : r0 + RPC],
                "outputs": o16[r0 : r0 + RPC],
                "diff": d16[r0 : r0 + RPC],
                "otgt": otgt[r0 : r0 + RPC].reshape(RB, P, 1),
            }
        )
    return in_maps


_NC_CACHE = {}


def _get_nc():
    if "nc" not in _NC_CACHE:
        _NC_CACHE["nc"] = build_nc()
    return _NC_CACHE["nc"]


def run(outputs, teacher_outputs, targets, trace=False, tmpdir=None):
    """Run on hardware; returns (per_sample[2048], BassKernelResults)."""
    from concourse.bass_utils import run_bass_kernel_spmd

    nc = _get_nc()
    in_maps = make_in_maps(outputs, teacher_outputs, targets)
    res = run_bass_kernel_spmd(
        nc, in_maps, core_ids=list(range(N_CORES)), trace=trace, tmpdir=tmpdir
    )
    per_sample = np.concatenate([r["loss"].reshape(-1) for r in res.results])
    return per_sample, res


def kernel(outputs, teacher_outputs, targets):
    per_sample, _ = run(outputs, teacher_outputs, targets)
    return np.float32(per_sample.mean(dtype=np.float64))


# revision 9
# speedup vs baseline: 1.2922x; 1.1070x over previous
"""Adaptive weighted knowledge-distillation loss on 8 TRN2 NeuronCores.

Pure data parallel: the batch (2048 rows) is split into 8 shards of 256
rows; each core streams its [256, 50257] shard and computes per-row
reductions over the class axis; the host averages the gathered [2048]
per-sample losses.

Inputs are uploaded as bf16 (tolerance is 2e-2; bf16 end-to-end error is
~2e-5), which halves HBM traffic. A third bf16 tensor d = t - o is
prepared on the host because the KL cross term only needs
D = sum(exp(t/4) * (t - o)); this removes one full fused product pass.
The per-row o[target] values are gathered on the host (f32, exact) and
uploaded, replacing the indirect-DMA gather.

Per-core math (row t = teacher logits, o = student logits, T = 4):
    zt4 = sum e^{t/4}   zt1 = sum e^t     zo4 = sum e^{o/4}  zo1 = sum e^o
    D   = sum e^{t/4} (t-o)               dt1 = sum t e^t
    H     = log zt1 - dt1/zt1
    alpha = clip(1 - H/log C, 0, 1)
    ce    = log zo1 - o[tgt]
    kl    = D/(4 zt4) - log zt4 + log zo4
    loss  = (1-alpha) ce + 16 alpha kl
No max-subtraction is needed: logits are standard-normal, exp() stays
comfortably inside f32/bf16 range.

Engine budget (measured rates, per core): ScalarE activation runs 1
elem/cycle/lane at any dtype (83.8us per full pass); plain
tensor_tensor bf16 runs 2x on DVE (52.4us); the stock fused
product+row-sum ops only run 1x (104.7us), so dve2x.py registers a
custom DVE op with a hand-authored 2X_1PORT uop program whose running
fold lands in the last even output element (mul_total, 52.4us/pass).
Work split:
  ScalarE (2 passes): e4t (zt4 accum), e4o (zo4 accum)
  VectorE (4 fused 2x passes): mul_total(e4t, d) -> D,
           pow4mul_total(e4t, t) -> dt1 = sum t e^t,
           pow4mul_total(e4t, ones) -> zt1 = sum e^t,
           pow4mul_total(e4o, ones) -> zo1 = sum e^o
The odd-width warm-up tile uses the 1x hardware-accumulator path;
every other width is even so the 2x programs engage.
"""

import sys

import numpy as np

try:
    import concourse  # noqa: F401
except ImportError:  # platform checkout location in the bench containers
    sys.path.insert(0, "/opt/trn_rl_repo")

import ml_dtypes

BF16 = ml_dtypes.bfloat16

B, C = 2048, 50257
N_CORES = 8
RPC = B // N_CORES  # rows per core = 256
P = 128  # SBUF partitions
RB = RPC // P  # row blocks per core = 2
W = 6144  # column tile width
LN_C = float(np.log(np.float32(C)))


def build_nc(rows=RPC, n_classes=C, w=W, debug=False):
    """Build the per-core Tile kernel (same SPMD graph for all cores)."""
    from contextlib import ExitStack

    import concourse.bacc as bacc
    import concourse.tile as tile
    from concourse import mybir

    import dve2x

    f32 = mybir.dt.float32
    bf16 = mybir.dt.bfloat16
    rb_count = rows // P
    assert rows % P == 0
    ln_c = float(np.log(np.float32(n_classes)))

    nc = bacc.Bacc("TRN2", target_bir_lowering=False, debug=debug)

    tch_ext = nc.declare_dram_parameter("teacher", [rows, n_classes], bf16, isOutput=False)
    outs_ext = nc.declare_dram_parameter("outputs", [rows, n_classes], bf16, isOutput=False)
    diff_ext = nc.declare_dram_parameter("diff", [rows, n_classes], bf16, isOutput=False)
    otgt_ext = nc.declare_dram_parameter("otgt", [rb_count, P, 1], f32, isOutput=False)
    loss_ext = nc.declare_dram_parameter("loss", [rb_count, P, 1], f32, isOutput=True)

    # Column tile schedule: first tile split small (odd - it takes the 1x
    # path and doubles as the pipeline warm-up), every other tile even so
    # the 2x DVE program engages; sums to n_classes exactly.
    n_full = n_classes // w - 1
    head = n_classes - n_full * w
    h1 = w // 4 + 1  # odd, 1x path, doubles as pipeline warm-up
    h2 = head - h1
    q1 = (h2 // 2) & ~1
    # small even tiles at the end so the engine pipeline drains quickly
    widths = [h1, q1, h2 - q1] + [w] * (n_full - 1) + [w // 2, w - w // 2]
    assert sum(widths) == n_classes
    assert all(x % 2 == 0 for x in widths[1:]) and all(x <= w for x in widths)
    nt = len(widths)

    with tile.TileContext(nc) as tc, ExitStack() as ctx:
        t_pool = ctx.enter_context(tc.tile_pool(name="t_in", bufs=3))
        o_pool = ctx.enter_context(tc.tile_pool(name="o_in", bufs=3))
        d_pool = ctx.enter_context(tc.tile_pool(name="d_in", bufs=2))
        e4t_pool = ctx.enter_context(tc.tile_pool(name="e4t", bufs=2))
        e4o_pool = ctx.enter_context(tc.tile_pool(name="e4o", bufs=2))
        sv_pool = ctx.enter_context(tc.tile_pool(name="scr_v", bufs=1))
        small = ctx.enter_context(tc.tile_pool(name="small", bufs=1))

        mult = mybir.AluOpType.mult
        add = mybir.AluOpType.add
        sub = mybir.AluOpType.subtract
        Exp = mybir.ActivationFunctionType.Exp
        Ln = mybir.ActivationFunctionType.Ln
        X = mybir.AxisListType.X

        # per-row-block accumulators: one column per column-tile
        QUANT = ("zt4", "zt1", "zo4", "zo1", "D", "dt1")
        acc = {}
        for rb in range(rb_count):
            for q in QUANT:
                acc[(rb, q)] = small.tile(
                    [P, nt], f32, tag=f"acc_{q}_{rb}", name=f"acc_{q}_{rb}"
                )

        otgt_sb = small.tile([P, rb_count], f32, tag="otgt", name="otgt")
        for rb in range(rb_count):
            nc.sync.dma_start(out=otgt_sb[:, rb : rb + 1], in_=otgt_ext[rb])

        ones = small.tile([P, w], bf16, tag="ones", name="ones")
        nc.gpsimd.memset(ones[:, :], 1.0)

        def emit_rb(rb):
            r0 = rb * P
            c0 = 0
            for ci, cw in enumerate(widths):
                t_tile = t_pool.tile([P, w], bf16, tag="t_in")
                o_tile = o_pool.tile([P, w], bf16, tag="o_in")
                d_tile = d_pool.tile([P, w], bf16, tag="d_in")
                nc.sync.dma_start(out=t_tile[:, :cw], in_=tch_ext[r0 : r0 + P, c0 : c0 + cw])
                nc.sync.dma_start(out=o_tile[:, :cw], in_=outs_ext[r0 : r0 + P, c0 : c0 + cw])
                nc.sync.dma_start(out=d_tile[:, :cw], in_=diff_ext[r0 : r0 + P, c0 : c0 + cw])

                e4t = e4t_pool.tile([P, w], bf16, tag="e4t")
                e4o = e4o_pool.tile([P, w], bf16, tag="e4o")

                # ScalarE: the only two exp passes, each with a free
                # row-sum accum (zt4, zo4)
                nc.scalar.activation(
                    e4t[:, :cw], t_tile[:, :cw], Exp, scale=0.25,
                    accum_out=acc[(rb, "zt4")][:, ci : ci + 1],
                )
                nc.scalar.activation(
                    e4o[:, :cw], o_tile[:, :cw], Exp, scale=0.25,
                    accum_out=acc[(rb, "zo4")][:, ci : ci + 1],
                )

                scr_v = sv_pool.tile([P, w], bf16, tag="scr_v")
                if cw % 2 == 0:
                    # 2x fused passes; totals extracted from the running fold
                    dve2x.mul_total(nc, out=scr_v[:, :cw], in0=e4t[:, :cw],
                                    in1=d_tile[:, :cw],
                                    total_out=acc[(rb, "D")][:, ci : ci + 1], cw=cw)
                    dve2x.pow4mul_total(nc, out=scr_v[:, :cw], in0=e4t[:, :cw],
                                        in1=t_tile[:, :cw],
                                        total_out=acc[(rb, "dt1")][:, ci : ci + 1], cw=cw)
                    dve2x.pow4mul_total(nc, out=scr_v[:, :cw], in0=e4t[:, :cw],
                                        in1=ones[:, :cw],
                                        total_out=acc[(rb, "zt1")][:, ci : ci + 1], cw=cw)
                    dve2x.pow4mul_total(nc, out=scr_v[:, :cw], in0=e4o[:, :cw],
                                        in1=ones[:, :cw],
                                        total_out=acc[(rb, "zo1")][:, ci : ci + 1], cw=cw)
                else:
                    # odd warm-up tile: 1x hardware-accumulator path
                    dve2x.mul_acc(nc, out=scr_v[:, :cw], in0=e4t[:, :cw],
                                  in1=d_tile[:, :cw],
                                  accum_out=acc[(rb, "D")][:, ci : ci + 1])
                    dve2x.pow4mul_acc(nc, out=scr_v[:, :cw], in0=e4t[:, :cw],
                                      in1=t_tile[:, :cw],
                                      accum_out=acc[(rb, "dt1")][:, ci : ci + 1])
                    dve2x.pow4mul_acc(nc, out=scr_v[:, :cw], in0=e4t[:, :cw],
                                      in1=ones[:, :cw],
                                      accum_out=acc[(rb, "zt1")][:, ci : ci + 1])
                    dve2x.pow4mul_acc(nc, out=scr_v[:, :cw], in0=e4o[:, :cw],
                                      in1=ones[:, :cw],
                                      accum_out=acc[(rb, "zo1")][:, ci : ci + 1])
                c0 += cw

        def emit_epilogue():
            # collapse per-tile partials; column r of each res tile = row
            # block r, so the whole scalar tail is one short op chain
            nrb = rb_count
            res = {}
            for q in QUANT:
                res[q] = small.tile([P, nrb], f32, tag=f"res_{q}", name=f"res_{q}")
                for rb in range(nrb):
                    nc.vector.tensor_reduce(
                        out=res[q][:, rb : rb + 1], in_=acc[(rb, q)][:, :nt],
                        axis=X, op=add,
                    )
            # lse tile: [zt4 | zt1 | zo4 | zo1] x rb  (one Ln instruction)
            zcat = small.tile([P, 4 * nrb], f32, tag="zcat", name="zcat")
            for qi, q in enumerate(("zt4", "zt1", "zo4", "zo1")):
                nc.vector.tensor_copy(
                    out=zcat[:, qi * nrb : (qi + 1) * nrb], in_=res[q][:, :]
                )
            lse = small.tile([P, 4 * nrb], f32, tag="lse", name="lse")
            nc.scalar.activation(lse[:, :], zcat[:, :], Ln)
            l_zt4 = lse[:, 0 * nrb : 1 * nrb]
            l_zt1 = lse[:, 1 * nrb : 2 * nrb]
            l_zo4 = lse[:, 2 * nrb : 3 * nrb]
            l_zo1 = lse[:, 3 * nrb : 4 * nrb]
            rcp = small.tile([P, 2 * nrb], f32, tag="rcp", name="rcp")
            nc.vector.reciprocal(out=rcp[:, : 2 * nrb], in_=zcat[:, : 2 * nrb])
            r_zt4 = rcp[:, 0 * nrb : 1 * nrb]
            r_zt1 = rcp[:, 1 * nrb : 2 * nrb]

            tmp = small.tile([P, 4 * nrb], f32, tag="tmp", name="tmp")
            a_ = tmp[:, 0 * nrb : 1 * nrb]
            ce = tmp[:, 1 * nrb : 2 * nrb]
            kl = tmp[:, 2 * nrb : 3 * nrb]
            t3 = tmp[:, 3 * nrb : 4 * nrb]
            # alpha = clip(1 - (log zt1 - dt1/zt1)/lnC, 0, 1)
            nc.vector.tensor_tensor(a_, res["dt1"][:, :], r_zt1, op=mult)
            nc.vector.tensor_tensor(a_, l_zt1, a_, op=sub)
            nc.vector.tensor_scalar(a_, a_, -1.0 / ln_c, 1.0, op0=mult, op1=add)
            nc.vector.tensor_scalar(
                a_, a_, 0.0, 1.0,
                op0=mybir.AluOpType.max, op1=mybir.AluOpType.min,
            )
            # ce = log(zo1) - o[tgt]
            nc.vector.tensor_tensor(ce, l_zo1, otgt_sb[:, :], op=sub)
            # kl = D*0.25/zt4 + (log zo4 - log zt4)
            nc.vector.tensor_tensor(kl, res["D"][:, :], r_zt4, op=mult)
            nc.vector.tensor_scalar(kl, kl, 0.25, None, op0=mult)
            nc.vector.tensor_tensor(t3, l_zo4, l_zt4, op=sub)
            nc.vector.tensor_tensor(kl, kl, t3, op=add)
            # loss = ce + alpha*(16*kl - ce)
            nc.vector.tensor_scalar(kl, kl, 16.0, None, op0=mult)
            nc.vector.tensor_tensor(kl, kl, ce, op=sub)
            loss_sb = small.tile([P, nrb], f32, tag="loss", name="loss")
            nc.vector.tensor_tensor(loss_sb[:, :], a_, kl, op=mult)
            nc.vector.tensor_tensor(loss_sb[:, :], loss_sb[:, :], ce, op=add)
            for rb in range(nrb):
                nc.sync.dma_start(out=loss_ext[rb], in_=loss_sb[:, rb : rb + 1])

        for rb in range(rb_count):
            emit_rb(rb)
        emit_epilogue()

    nc.compile()
    dve2x.enable_2x_on_module(nc)
    return nc


def make_in_maps(outputs, teacher_outputs, targets):
    outputs = np.ascontiguousarray(outputs, dtype=np.float32)
    teacher = np.ascontiguousarray(teacher_outputs, dtype=np.float32)
    tgt = np.asarray(targets).astype(np.int64).reshape(-1)
    t16 = teacher.astype(BF16)
    o16 = outputs.astype(BF16)
    d16 = (teacher - outputs).astype(BF16)
    otgt = outputs[np.arange(B), tgt].astype(np.float32)
    in_maps = []
    for i in range(N_CORES):
        r0 = i * RPC
        in_maps.append(
            {
                "teacher": t16[r0 : r0 + RPC],
                "outputs": o16[r0 : r0 + RPC],
                "diff": d16[r0 : r0 + RPC],
                "otgt": otgt[r0 : r0 + RPC].reshape(RB, P, 1),
            }
        )
    return in_maps


_NC_CACHE = {}


def _get_nc():
    if "nc" not in _NC_CACHE:
        _NC_CACHE["nc"] = build_nc()
    return _NC_CACHE["nc"]


def run(outputs, teacher_outputs, targets, trace=False, tmpdir=None):
    """Run on hardware; returns (per_sample[2048], BassKernelResults)."""
    from concourse.bass_utils import run_bass_kernel_spmd

    nc = _get_nc()
    in_maps = make_in_maps(outputs, teacher_outputs, targets)
    res = run_bass_kernel_spmd(
        nc, in_maps, core_ids=list(range(N_CORES)), trace=trace, tmpdir=tmpdir
    )
    per_sample = np.concatenate([r["loss"].reshape(-1) for r in res.results])
    return per_sample, res


def kernel(outputs, teacher_outputs, targets):
    per_sample, _ = run(outputs, teacher_outputs, targets)
    return np.float32(per_sample.mean(dtype=np.float64))
